# revision 8
# baseline (speedup 1.0000x reference)
"""Trainium2 Bass kernel for nn_DSP_33131377721365.

reference math (x: [4, 32, 720, 720] f32, conv_w: [32, 32, 3, 1] f32):
  s[b,h,w]    = sum_c x[b,c,h,w]
  d[b,h,w]    = (1/9) * sum_{t=0..8} s[b, h+t-4, w+t-4]   (zero padded)
  out[b,o,h,w]= sum_{j=0..2} wsum[o,j] * d[b, h-1+j, w]   (zero padded)
  where wsum[o,j] = sum_c conv_w[o,c,j,0]

Sharding: 8 cores = 4 batches x 2 H-halves (360 rows each), data parallel;
host pre-pads each shard with 5 halo rows (pool 4 + conv 1) so the device
program is uniform SPMD. Inside a core: 4 H-blocks of 90 output rows, H rows
on SBUF partitions, W on the free dim. The 9-tap diagonal is factored as
t = 3a + b; partition shifts (which compute engines cannot address) are done
with SBUF->SBUF DMA copies, free-dim shifts with skewed strided APs. The
3-tap H conv + broadcast to the 32 output channels runs on the TensorEngine
as a banded matmul per output channel (the 1/9 and the conv weights are
folded into the band matrix), accumulated in PSUM and evacuated by ScalarE.
"""

import numpy as np

import concourse.bass as bass
import concourse.bacc as bacc
import concourse.mybir as mybir
import concourse.tile as tile
from concourse.bass_utils import run_bass_kernel_spmd

FP = mybir.dt.float32
KTAPS = 9
PADW = KTAPS // 2  # 4
HALO = PADW + 1    # 5

B, C, H, W = 4, 32, 720, 720
O = 32
N_CORES = 8
HS = H // 2        # 360 output rows per core
BLK = 90           # output rows per block
NBLK = HS // BLK   # 4
KP = BLK + 2
EROWS = BLK + 8
SROWS = BLK + 2 * HALO
CH = C // 2
WP = W + 2 * PADW
WE = W + 6


def _skew_ap(t, P, W_out, inner_pitch, shift):
    ap = t[:]
    pstride = ap.ap[0][0]
    return bass.AP(ap.tensor, ap.offset,
                   [[pstride, P], [1, W_out], [inner_pitch + shift, 3]])


def _build(nc, wchunk=512):
    xs = nc.declare_dram_parameter("xs", [HS + 2 * HALO, C, W], FP, isOutput=False)
    am = nc.declare_dram_parameter("amat", [KP, O, BLK], FP, isOutput=False)
    mk = nc.declare_dram_parameter("mask", [KP, NBLK], FP, isOutput=False)
    out = nc.declare_dram_parameter("out", [O, HS, W], FP, isOutput=True)

    add = mybir.AluOpType.add
    mult = mybir.AluOpType.mult

    with tile.TileContext(nc) as tc:
        with (
            tc.tile_pool(name="xa", bufs=3) as xpool,
            tc.tile_pool(name="sp", bufs=3) as sppool,
            tc.tile_pool(name="sh", bufs=2) as shpool,
            tc.tile_pool(name="dt", bufs=2) as dpool,
            tc.tile_pool(name="cst", bufs=1) as cpool,
            tc.tile_pool(name="ob", bufs=3) as opool,
            tc.tile_pool(name="ps", bufs=4, space="PSUM") as pspool,
        ):
            amt = cpool.tile([KP, O, BLK], FP)
            nc.sync.dma_start(amt[:], am[:])
            mkt = cpool.tile([KP, NBLK], FP)
            nc.sync.dma_start(mkt[:], mk[:])

            for blk in range(NBLK):
                r0 = blk * BLK
                xa = xpool.tile([SROWS, CH, W], FP, tag="x")
                xb = xpool.tile([SROWS, CH, W], FP, tag="x")
                nc.sync.dma_start(xa[:], xs[r0:r0 + SROWS, 0:CH, :])
                nc.sync.dma_start(xb[:], xs[r0:r0 + SROWS, CH:C, :])

                sp = sppool.tile([SROWS, WP], FP, tag="s")
                sp2 = sppool.tile([SROWS, W], FP, tag="s")
                nc.vector.memset(sp[:, 0:PADW], 0.0)
                nc.vector.memset(sp[:, PADW + W:WP], 0.0)
                nc.vector.tensor_reduce(
                    out=sp[:, PADW:PADW + W],
                    in_=xa[:].rearrange("p c w -> p w c"),
                    axis=mybir.AxisListType.X, op=add,
                )
                nc.vector.tensor_reduce(
                    out=sp2[:],
                    in_=xb[:].rearrange("p c w -> p w c"),
                    axis=mybir.AxisListType.X, op=add,
                )
                nc.vector.tensor_tensor(
                    out=sp[:, PADW:PADW + W], in0=sp[:, PADW:PADW + W],
                    in1=sp2[:], op=add,
                )

                # two-level 9-tap diagonal (t = 3a + b)
                spx3 = shpool.tile([EROWS, 3, WP], FP, tag="sh")
                for b in range(3):
                    nc.sync.dma_start(spx3[:, b, :], sp[b:b + EROWS, :])
                e = sppool.tile([EROWS, WE], FP, tag="s")
                nc.vector.tensor_reduce(
                    out=e[:], in_=_skew_ap(spx3, EROWS, WE, WP, 1),
                    axis=mybir.AxisListType.X, op=add,
                )
                ex3 = shpool.tile([KP, 3, WE], FP, tag="sh")
                for a in range(3):
                    nc.sync.dma_start(ex3[:, a, :], e[3 * a:3 * a + KP, :])
                d = dpool.tile([KP, W], FP)
                nc.vector.tensor_reduce(
                    out=d[:], in_=_skew_ap(ex3, KP, W, WE, 3),
                    axis=mybir.AxisListType.X, op=add,
                )
                # zero out-of-image d rows (conv zero padding at global edges)
                nc.vector.tensor_scalar(
                    out=d[:], in0=d[:], scalar1=mkt[:, blk:blk + 1],
                    scalar2=None, op0=mult,
                )

                for o in range(O):
                    ps = pspool.tile([BLK, W], FP)
                    for w0 in range(0, W, wchunk):
                        w1 = min(w0 + wchunk, W)
                        nc.tensor.matmul(
                            ps[:, w0:w1], amt[:, o, :], d[:, w0:w1],
                            start=True, stop=True,
                        )
                    ob = opool.tile([BLK, W], FP)
                    nc.scalar.copy(out=ob[:], in_=ps[:])
                    nc.sync.dma_start(out[o, r0:r0 + BLK, :], ob[:])
    return nc


def _make_amat(conv_w):
    wsum9 = conv_w.sum(axis=1)[:, :, 0].astype(np.float64) / KTAPS  # [O, 3]
    A = np.zeros((KP, O, BLK), np.float32)
    for j in range(3):
        for m in range(BLK):
            A[m + j, :, m] = wsum9[:, j].astype(np.float32)
    return A


def _make_mask(core_h0):
    mask = np.zeros((KP, NBLK), np.float32)
    for b in range(NBLK):
        for q in range(KP):
            g = core_h0 + b * BLK - 1 + q
            mask[q, b] = 1.0 if 0 <= g < H else 0.0
    return mask


def _make_shard(xt_b, h0):
    """xt_b: [H, C, W] one batch (h-major). Returns padded [HS+10, C, W]."""
    sh = np.zeros((HS + 2 * HALO, C, W), np.float32)
    lo, hi = h0 - HALO, h0 + HS + HALO
    slo, shi = max(lo, 0), min(hi, H)
    sh[slo - lo:shi - lo] = xt_b[slo:shi]
    return sh


def _run(x, conv_w, trace=False, **spmd_kwargs):
    x = np.ascontiguousarray(np.asarray(x, dtype=np.float32))
    conv_w = np.asarray(conv_w, dtype=np.float32)
    assert x.shape == (B, C, H, W) and conv_w.shape == (O, C, 3, 1)

    nc = bacc.Bacc("TRN2", target_bir_lowering=False, debug=False,
                   num_devices=N_CORES)
    _build(nc)
    nc.compile()

    amat = _make_amat(conv_w)
    xt = np.ascontiguousarray(x.transpose(0, 2, 1, 3))  # [B, H, C, W]
    in_maps = []
    for i in range(N_CORES):
        b, half = i // 2, i % 2
        h0 = half * HS
        in_maps.append({
            "xs": _make_shard(xt[b], h0),
            "amat": amat,
            "mask": _make_mask(h0),
        })
    res = run_bass_kernel_spmd(nc, in_maps, list(range(N_CORES)),
                               trace=trace, **spmd_kwargs)
    out = np.empty((B, O, H, W), np.float32)
    for i in range(N_CORES):
        b, half = i // 2, i % 2
        out[b, :, half * HS:(half + 1) * HS, :] = res.results[i]["out"]
    return out, res


def kernel(x, conv_w):
    out, _ = _run(x, conv_w, trace=False)
    return out


# revision 10
# speedup vs baseline: 414.0647x; 414.0647x over previous
"""Trainium2 Bass kernel for nn_DSP_33131377721365.

reference math (x: [4, 32, 720, 720] f32, conv_w: [32, 32, 3, 1] f32):
  s[b,h,w]    = sum_c x[b,c,h,w]
  d[b,h,w]    = (1/9) * sum_{t=0..8} s[b, h+t-4, w+t-4]   (zero padded)
  out[b,o,h,w]= sum_{j=0..2} wsum[o,j] * d[b, h-1+j, w]   (zero padded)
  where wsum[o,j] = sum_c conv_w[o,c,j,0]

Sharding: 8 cores = 4 batches x 2 H-halves (360 rows each), data parallel;
host pre-pads each shard with 5 halo rows (pool 4 + conv 1) so the device
program is uniform SPMD. Inside a core: 4 H-blocks of 90 output rows, H rows
on SBUF partitions, W on the free dim. The 9-tap diagonal is factored as
t = 3a + b; partition shifts (which compute engines cannot address) are done
with SBUF->SBUF DMA copies, free-dim shifts with skewed strided APs. The
3-tap H conv + broadcast to the 32 output channels runs on the TensorEngine
as a banded matmul per output channel (the 1/9 and the conv weights are
folded into the band matrix), accumulated in PSUM and evacuated by ScalarE.
"""

import numpy as np

import concourse.bass as bass
import concourse.bacc as bacc
import concourse.mybir as mybir
import concourse.tile as tile
from concourse.bass_utils import run_bass_kernel_spmd

FP = mybir.dt.float32
KTAPS = 9
PADW = KTAPS // 2  # 4
HALO = PADW + 1    # 5

B, C, H, W = 4, 32, 720, 720
O = 32
N_CORES = 8
HS = H // 2        # 360 output rows per core
BLK = 90           # output rows per block
NBLK = HS // BLK   # 4
KP = BLK + 2
EROWS = BLK + 8
SROWS = BLK + 2 * HALO
CH = C // 2
WP = W + 2 * PADW
WE = W + 6


def _skew_ap(t, P, W_out, inner_pitch, shift):
    ap = t[:]
    pstride = ap.ap[0][0]
    return bass.AP(ap.tensor, ap.offset,
                   [[pstride, P], [1, W_out], [inner_pitch + shift, 3]])


def _build(nc, wchunk=512, reps=1):
    xs = nc.declare_dram_parameter("xs", [HS + 2 * HALO, C, W], FP, isOutput=False)
    am = nc.declare_dram_parameter("amat", [KP, O, BLK], FP, isOutput=False)
    mk = nc.declare_dram_parameter("mask", [KP, NBLK], FP, isOutput=False)
    out = nc.declare_dram_parameter("out", [O, HS, W], FP, isOutput=True)

    add = mybir.AluOpType.add
    mult = mybir.AluOpType.mult

    with tile.TileContext(nc) as tc:
        with (
            tc.tile_pool(name="xa", bufs=3) as xpool,
            tc.tile_pool(name="sp", bufs=3) as sppool,
            tc.tile_pool(name="sh", bufs=2) as shpool,
            tc.tile_pool(name="dt", bufs=2) as dpool,
            tc.tile_pool(name="cst", bufs=1) as cpool,
            tc.tile_pool(name="ob", bufs=3) as opool,
            tc.tile_pool(name="ps", bufs=4, space="PSUM") as pspool,
        ):
            amt = cpool.tile([KP, O, BLK], FP)
            nc.sync.dma_start(amt[:], am[:])
            mkt = cpool.tile([KP, NBLK], FP)
            nc.sync.dma_start(mkt[:], mk[:])

            for blk in range(NBLK * reps):
                blk = blk % NBLK
                r0 = blk * BLK
                xa = xpool.tile([SROWS, CH, W], FP, tag="x")
                xb = xpool.tile([SROWS, CH, W], FP, tag="x")
                nc.sync.dma_start(xa[:], xs[r0:r0 + SROWS, 0:CH, :])
                nc.sync.dma_start(xb[:], xs[r0:r0 + SROWS, CH:C, :])

                sp = sppool.tile([SROWS, WP], FP, tag="s")
                sp2 = sppool.tile([SROWS, W], FP, tag="s")
                nc.vector.memset(sp[:, 0:PADW], 0.0)
                nc.vector.memset(sp[:, PADW + W:WP], 0.0)
                nc.vector.tensor_reduce(
                    out=sp[:, PADW:PADW + W],
                    in_=xa[:].rearrange("p c w -> p w c"),
                    axis=mybir.AxisListType.X, op=add,
                )
                nc.vector.tensor_reduce(
                    out=sp2[:],
                    in_=xb[:].rearrange("p c w -> p w c"),
                    axis=mybir.AxisListType.X, op=add,
                )
                nc.vector.tensor_tensor(
                    out=sp[:, PADW:PADW + W], in0=sp[:, PADW:PADW + W],
                    in1=sp2[:], op=add,
                )

                # two-level 9-tap diagonal (t = 3a + b)
                spx3 = shpool.tile([EROWS, 3, WP], FP, tag="sh")
                for b in range(3):
                    nc.sync.dma_start(spx3[:, b, :], sp[b:b + EROWS, :])
                e = sppool.tile([EROWS, WE], FP, tag="s")
                nc.vector.tensor_reduce(
                    out=e[:], in_=_skew_ap(spx3, EROWS, WE, WP, 1),
                    axis=mybir.AxisListType.X, op=add,
                )
                ex3 = shpool.tile([KP, 3, WE], FP, tag="sh")
                for a in range(3):
                    nc.sync.dma_start(ex3[:, a, :], e[3 * a:3 * a + KP, :])
                d = dpool.tile([KP, W], FP)
                nc.vector.tensor_reduce(
                    out=d[:], in_=_skew_ap(ex3, KP, W, WE, 3),
                    axis=mybir.AxisListType.X, op=add,
                )
                # zero out-of-image d rows (conv zero padding at global edges)
                nc.vector.tensor_scalar(
                    out=d[:], in0=d[:], scalar1=mkt[:, blk:blk + 1],
                    scalar2=None, op0=mult,
                )

                for o in range(O):
                    ps = pspool.tile([BLK, W], FP)
                    for w0 in range(0, W, wchunk):
                        w1 = min(w0 + wchunk, W)
                        nc.tensor.matmul(
                            ps[:, w0:w1], amt[:, o, :], d[:, w0:w1],
                            start=True, stop=True,
                        )
                    ob = opool.tile([BLK, W], FP)
                    nc.scalar.copy(out=ob[:], in_=ps[:])
                    nc.sync.dma_start(out[o, r0:r0 + BLK, :], ob[:])
    return nc


def _make_amat(conv_w):
    wsum9 = conv_w.sum(axis=1)[:, :, 0].astype(np.float64) / KTAPS  # [O, 3]
    A = np.zeros((KP, O, BLK), np.float32)
    for j in range(3):
        for m in range(BLK):
            A[m + j, :, m] = wsum9[:, j].astype(np.float32)
    return A


def _make_mask(core_h0):
    mask = np.zeros((KP, NBLK), np.float32)
    for b in range(NBLK):
        for q in range(KP):
            g = core_h0 + b * BLK - 1 + q
            mask[q, b] = 1.0 if 0 <= g < H else 0.0
    return mask


def _make_shard(xt_b, h0):
    """xt_b: [H, C, W] one batch (h-major). Returns padded [HS+10, C, W]."""
    sh = np.zeros((HS + 2 * HALO, C, W), np.float32)
    lo, hi = h0 - HALO, h0 + HS + HALO
    slo, shi = max(lo, 0), min(hi, H)
    sh[slo - lo:shi - lo] = xt_b[slo:shi]
    return sh


def _run(x, conv_w, trace=False, **spmd_kwargs):
    x = np.ascontiguousarray(np.asarray(x, dtype=np.float32))
    conv_w = np.asarray(conv_w, dtype=np.float32)
    assert x.shape == (B, C, H, W) and conv_w.shape == (O, C, 3, 1)

    nc = bacc.Bacc("TRN2", target_bir_lowering=False, debug=False,
                   num_devices=N_CORES)
    _build(nc)
    nc.compile()

    amat = _make_amat(conv_w)
    xt = np.ascontiguousarray(x.transpose(0, 2, 1, 3))  # [B, H, C, W]
    in_maps = []
    for i in range(N_CORES):
        b, half = i // 2, i % 2
        h0 = half * HS
        in_maps.append({
            "xs": _make_shard(xt[b], h0),
            "amat": amat,
            "mask": _make_mask(h0),
        })
    res = run_bass_kernel_spmd(nc, in_maps, list(range(N_CORES)),
                               trace=trace, **spmd_kwargs)
    out = np.empty((B, O, H, W), np.float32)
    for i in range(N_CORES):
        b, half = i // 2, i % 2
        out[b, :, half * HS:(half + 1) * HS, :] = res.results[i]["out"]
    return out, res


def kernel(x, conv_w):
    out, _ = _run(x, conv_w, trace=False)
    return out
